# revision 40
# baseline (speedup 1.0000x reference)
"""DGI (Deep Graph Infomax) Trainium2 kernel.

Strategy (8 NeuronCores, one shared SPMD program):
  - Host packs xc = [x | x[perm]] and uploads it TRANSPOSED (xcT [512, N] f16)
    so phase 1 needs no DMA transposes.  Dst nodes are LPT-balanced by degree
    into (core, tile, slot) buckets to minimize max-over-core gather padding.
  - Phase 1 (replicated): xwc[v] = dinv[v] * [x[v]@W | x[perm[v]]@W] (f16,
    1024B rows) written to three DRAM region tables (_BASES) so round-r
    gathers start as soon as phase 1 completes region r.  dinv[s] folded
    here makes the aggregation one-hots BINARY.
  - Aggregation (dst-sharded, both passes fused): per (dst tile, region) ONE
    dma_gather of 1024B rows serves z1 AND z2.  num_idxs is the exact
    16-rounded shared valid count (GpSimd's idx-unpack loop scales with
    STATIC num_idxs; trailing -1 idxs are skipped).  Per 128-edge tile:
    binary one-hot via is_equal on DVE, ONE 128x128x512 PE matmul into a
    single PSUM bank; rounds accumulate via SBUF (zacc).
  - Consumer: dinv[d]*PReLU fused as (a*c)*v + ((1-a)*c)*max(v,0) with the
    max branch as Relu on the ACT engine.  z1 column sums accumulate in
    PSUM via ones-vector matmuls as tiles finish.
  - summary sigmoid + 1KB AllReduce + wsum = disc_W @ summary (PE), then
    fused multiply+row-sum dots per dst tile on DVE.  Host unshards via the
    balanced assignment maps.
"""

import os

import numpy as np

_P = 128
_C = 8
# class/round boundaries (tile-aligned, each span <= 32768 for int16 idx);
# 3 rounds let gathers of round r start once phase 1 finished region r.
# Round 0 is small so gathers start early in phase 1.
_BASES = [0, 13312, 29952]
_NCLS = 3


def _assign_balanced(deg, C, DT):
    """LPT-balance dst nodes into C*DT buckets of <=128 by degree, so the
    max-over-cores gather padding shrinks.  Returns (coreof, dtof, slotof)."""
    import heapq

    N = deg.shape[0]
    NB = C * DT
    order = np.argsort(-deg, kind="stable")
    bcnt = np.zeros(NB, np.int64)
    assign = np.zeros(N, np.int64)
    heap = [(0.0, b) for b in range(NB)]
    heapq.heapify(heap)
    for v in order:
        while True:
            s, b = heapq.heappop(heap)
            if bcnt[b] < _P:
                break
        assign[v] = b
        bcnt[b] += 1
        heapq.heappush(heap, (s + float(deg[v]), b))
    slotof = np.zeros(N, np.int64)
    # slot within bucket: stable order of assignment
    border = np.argsort(assign, kind="stable")
    pos_in = np.arange(N) - np.concatenate(
        [[0], np.cumsum(np.bincount(assign, minlength=NB))[:-1]]
    )[assign[border]]
    slotof[border] = pos_in
    return assign // DT, assign % DT, slotof


def _build_streams(es, core, dt, dstl, C, DT):
    """Per-core gather index + local-dst streams with shared tile structure.

    Groups edges by (dst_tile, class); class = src region from _BASES.
    Pads with trailing idx=-1 (skipped) after an idx=0 fill to the shared
    valid count Mv.  Returns (idx_sbuf, dl_sbuf, Tmax, Mv, off, n_et).
    """
    bases = np.asarray(_BASES, np.int64)
    cls = (es[:, None] >= bases[None, 1:]).sum(axis=1)

    gid = (core * DT + dt) * _NCLS + cls
    NG = C * DT * _NCLS
    cnt = np.bincount(gid, minlength=NG).reshape(C, DT, _NCLS)
    T = -(-cnt // _P)
    Tmax = np.maximum(T.max(axis=0), 1)
    # shared valid count per group, 16-rounded: passed as the gather's
    # num_idxs (the firmware unpack loop scales with STATIC num_idxs, so
    # don't pad to 128-tile boundaries)
    Mv = 16 * (-(-np.maximum(cnt.max(axis=0), 1) // 16))
    flat = Tmax.reshape(-1)
    off = np.concatenate([[0], np.cumsum(flat)[:-1]]).reshape(DT, _NCLS)
    n_et = int(flat.sum())

    order = np.argsort(gid, kind="stable")
    sorted_gid = gid[order]
    g_starts = np.concatenate(
        [[0], np.cumsum(np.bincount(sorted_gid, minlength=NG))[:-1]]
    )
    rank = np.arange(order.size) - g_starts[sorted_gid]
    g_dt = (sorted_gid // _NCLS) % DT
    g_cls = sorted_gid % _NCLS
    pos = off[g_dt, g_cls] * _P + rank
    core_s = sorted_gid // (DT * _NCLS)

    L = n_et * _P
    # idx=-1 (firmware skips trailing negatives); dummy-valid idx=0 fill up
    # to the shared per-group count Mv so num_idxs_reg is core-invariant.
    idx16 = np.full((C, L), -1, np.int16)
    dl = np.full((C, L), -1.0, np.float16)
    es_s = es[order]
    idx16[core_s, pos] = (es_s - bases[g_cls]).astype(np.int16)
    dl[core_s, pos] = dstl[order].astype(np.float16)
    for dti in range(DT):
        for ci in range(_NCLS):
            base = off[dti, ci] * _P
            m = int(Mv[dti, ci])
            for c in range(C):
                v = int(cnt[c, dti, ci])
                if v < m:
                    idx16[c, base + v : base + m] = 0

    idx_w = idx16.reshape(C, L // 16, 16).transpose(0, 2, 1)
    idx_sbuf = np.ascontiguousarray(np.tile(idx_w, (1, 8, 1)))
    dl_sbuf = np.ascontiguousarray(dl.reshape(C, n_et, _P).transpose(0, 2, 1))
    return idx_sbuf, dl_sbuf, Tmax, Mv, off, n_et


def kernel(x, W, b, a, disc_W, edge_index, perm):
    import bass_rust
    import concourse.bacc as bacc
    import concourse.mybir as mybir
    import concourse.tile as tile
    from concourse.bass_utils import run_bass_kernel_spmd

    x = np.asarray(x)
    W = np.asarray(W)
    b = np.asarray(b, np.float32)
    a = np.asarray(a, np.float32)
    disc_W = np.asarray(disc_W, np.float32)
    ei = np.asarray(edge_index, np.int64)
    perm_np = np.asarray(perm, np.int64)

    N, F = x.shape
    H = W.shape[1]
    C = _C
    NS = N // C
    DT = -(-NS // _P)
    LAST = NS - (DT - 1) * _P
    NT = -(-N // _P)  # global node tiles (391)
    rbounds = _BASES + [N]  # region row boundaries
    f16 = mybir.dt.float16
    f32 = mybir.dt.float32
    has_bias = bool(np.any(b))

    # ---- host preprocessing -------------------------------------------
    src = ei[0]
    dst = ei[1]
    deg = (np.bincount(dst, minlength=N) + 1.0).astype(np.float32)
    dinv = (1.0 / np.sqrt(deg)).astype(np.float32)
    loops = np.arange(N, dtype=np.int64)
    es = np.concatenate([src, loops])
    ed = np.concatenate([dst, loops])

    coreof, dtof, slotof = _assign_balanced(deg, C, DT)
    i_s, d_s, Tm, Mv, Ot, n_et = _build_streams(
        es, coreof[ed], dtof[ed], slotof[ed], C, DT
    )
    maxT = int(Tm.max())
    a0 = float(a.reshape(-1)[0])

    xc = np.concatenate([x, x[perm_np]], axis=1).astype(np.float16)
    xcT = np.ascontiguousarray(xc.T)  # [2F, N]
    dwT = np.ascontiguousarray(disc_W.T.astype(np.float32))
    iota_np = np.tile(np.arange(_P, dtype=np.float16)[None, :], (_P, 1))

    # dinv in node-tile layout [128, NT] (pad 0)
    dinv_nt = np.zeros((_P, NT), np.float32)
    dinv_pad = np.zeros(NT * _P, np.float32)
    dinv_pad[:N] = dinv
    dinv_nt[:, :] = dinv_pad.reshape(NT, _P).T
    # per-core dst-tile layout [128, DT] (pad 0 for empty slots)
    dinv_dst = np.zeros((C, _P, DT), np.float32)
    dinv_dst[coreof, slotof, dtof] = dinv
    adinv_dst = a0 * dinv_dst  # for fused c*PReLU(v) = (a*c)v + ((1-a)c)max(v,0)
    c1_dst = (1.0 - a0) * dinv_dst

    # ---- device program -----------------------------------------------
    nc = bacc.Bacc("TRN2", target_bir_lowering=False, debug=False, num_devices=C)

    t_xcT = nc.dram_tensor("xcT", [2 * F, N], f16, kind="ExternalInput")
    t_W = nc.dram_tensor("w32", [F, H], f32, kind="ExternalInput")
    t_b = nc.dram_tensor("bvec", [H], f32, kind="ExternalInput")
    t_a = nc.dram_tensor("avec", [1], f32, kind="ExternalInput")
    t_dwT = nc.dram_tensor("dwT", [H, H], f32, kind="ExternalInput")
    t_iota = nc.dram_tensor("iota", [_P, _P], f16, kind="ExternalInput")
    t_ident = nc.dram_tensor("ident_in", [_P, _P], f32, kind="ExternalInput")
    t_dnt = nc.dram_tensor("dinv_nt", [_P, NT], f32, kind="ExternalInput")
    t_ddst = nc.dram_tensor("dinv_dst", [_P, DT], f32, kind="ExternalInput")
    t_adinv = nc.dram_tensor("adinv_dst", [_P, DT], f32, kind="ExternalInput")
    t_c1 = nc.dram_tensor("c1_dst", [_P, DT], f32, kind="ExternalInput")
    t_i = nc.dram_tensor("idx", [_P, n_et * 8], mybir.dt.int16, kind="ExternalInput")
    t_d = nc.dram_tensor("dstl", [_P, n_et], f16, kind="ExternalInput")

    t_pos = nc.dram_tensor("pos_out", [_P, DT], f32, kind="ExternalOutput")
    t_neg = nc.dram_tensor("neg_out", [_P, DT], f32, kind="ExternalOutput")

    t_xwc = [
        nc.dram_tensor(f"xwc_{r}", [rbounds[r + 1] - rbounds[r], 2 * H], f16)
        for r in range(_NCLS)
    ]
    t_ar_in = nc.dram_tensor("ar_in", [H], f32)
    t_ar_out = nc.dram_tensor("ar_out", [H], f32, addr_space="Shared")

    CHUNK = 512

    with tile.TileContext(nc) as tc:
        import contextlib

        ctx = contextlib.ExitStack()
        consts = ctx.enter_context(tc.tile_pool(name="consts", bufs=1))
        ph1 = ctx.enter_context(tc.tile_pool(name="ph1", bufs=3))
        ph1ps = ctx.enter_context(tc.tile_pool(name="ph1ps", bufs=2, space="PSUM"))
        gpool = ctx.enter_context(tc.tile_pool(name="gpool", bufs=4))
        stp = ctx.enter_context(tc.tile_pool(name="stp", bufs=8))
        aggps = ctx.enter_context(tc.tile_pool(name="aggps", bufs=2, space="PSUM"))
        misc = ctx.enter_context(tc.tile_pool(name="misc", bufs=2))
        miscps = ctx.enter_context(tc.tile_pool(name="miscps", bufs=1, space="PSUM"))
        sumps = ctx.enter_context(tc.tile_pool(name="sumps", bufs=1, space="PSUM"))

        # ---- constants ----
        W0 = consts.tile([_P, H], f16, tag="W0")
        W1 = consts.tile([_P, H], f16, tag="W1")
        W0f = consts.tile([_P, H], f32, tag="W0f")
        W1f = consts.tile([_P, H], f32, tag="W1f")
        nc.sync.dma_start(W0f[:], t_W[0:_P, :])
        nc.sync.dma_start(W1f[:], t_W[_P : 2 * _P, :])
        nc.vector.tensor_copy(W0[:], W0f[:])
        nc.vector.tensor_copy(W1[:], W1f[:])
        iota_t = consts.tile([_P, _P], f16, tag="iota")
        nc.sync.dma_start(iota_t[:], t_iota[:])
        b_sb = consts.tile([1, H], f32, tag="b_sb")
        nc.sync.dma_start(b_sb[:], t_b[None, :])
        a_sb = consts.tile([1, 1], f32, tag="a_sb")
        nc.sync.dma_start(a_sb[:], t_a[None, :])
        dwT0 = consts.tile([_P, H], f32, tag="dwT0")
        dwT1 = consts.tile([_P, H], f32, tag="dwT1")
        nc.sync.dma_start(dwT0[:], t_dwT[0:_P, :])
        nc.sync.dma_start(dwT1[:], t_dwT[_P : 2 * _P, :])
        dnt = consts.tile([_P, NT], f32, tag="dnt")
        nc.sync.dma_start(dnt[:], t_dnt[:])
        ddst = consts.tile([_P, DT], f32, tag="ddst")
        nc.sync.dma_start(ddst[:], t_ddst[:])
        adinv = consts.tile([_P, DT], f32, tag="adinv")
        nc.sync.dma_start(adinv[:], t_adinv[:])
        c1t = consts.tile([_P, DT], f32, tag="c1t")
        nc.sync.dma_start(c1t[:], t_c1[:])
        ones_row = consts.tile([1, _P], f32, tag="ones_row")
        nc.vector.memset(ones_row[:], 1.0)
        ones_col = consts.tile([_P, 1], f32, tag="ones_col")
        nc.vector.memset(ones_col[:], 1.0)

        # broadcasts via K=1 matmul
        bb_ps = miscps.tile([_P, H], f32, tag="mps")
        nc.tensor.matmul(bb_ps[:], ones_row[:], b_sb[:], start=True, stop=True)
        b_bc = consts.tile([_P, H], f32, tag="b_bc")
        nc.vector.tensor_copy(b_bc[:], bb_ps[:])
        ab_ps = miscps.tile([_P, 1], f32, tag="mps")
        nc.tensor.matmul(ab_ps[:], ones_row[:], a_sb[:], start=True, stop=True)
        a_bc = consts.tile([_P, 1], f32, tag="a_bc")
        nc.vector.tensor_copy(a_bc[:], ab_ps[:])

        # ---- stream loads ----
        i_sb = consts.tile([_P, n_et * 8], mybir.dt.int16, tag="i_sb")
        d_sb = consts.tile([_P, n_et], f16, tag="d_sb")
        nc.sync.dma_start(i_sb[:], t_i[:])
        nc.sync.dma_start(d_sb[:], t_d[:])

        # ---- phase 1: xwc = dinv * [x@W | x[perm]@W], lo tiles first ----
        def phase1_range(t0, t1):
            for tt0 in range(t0, t1, CHUNK // _P):
                ntiles = min(CHUNK // _P, t1 - tt0)
                cols = min(CHUNK, N - tt0 * _P)
                xt = []
                for fb in range(4):
                    xtb = ph1.tile([_P, CHUNK], f16, tag=f"xt{fb}")
                    nc.sync.dma_start(
                        xtb[:, :cols],
                        t_xcT[fb * _P : (fb + 1) * _P, tt0 * _P : tt0 * _P + cols],
                    )
                    xt.append(xtb)
                for o in range(ntiles):
                    ti = tt0 + o
                    m = min(_P, N - ti * _P)
                    psA = ph1ps.tile([_P, H], f32, tag="psA")
                    psB = ph1ps.tile([_P, H], f32, tag="psB")
                    nc.tensor.matmul(
                        psA[:m, :], xt[0][:, o * _P : o * _P + m], W0[:],
                        start=True, stop=False,
                    )
                    nc.tensor.matmul(
                        psA[:m, :], xt[1][:, o * _P : o * _P + m], W1[:],
                        start=False, stop=True,
                    )
                    nc.tensor.matmul(
                        psB[:m, :], xt[2][:, o * _P : o * _P + m], W0[:],
                        start=True, stop=False,
                    )
                    nc.tensor.matmul(
                        psB[:m, :], xt[3][:, o * _P : o * _P + m], W1[:],
                        start=False, stop=True,
                    )
                    xwc_sb = ph1.tile([_P, 2 * H], f16, tag="xwc_sb")
                    nc.scalar.activation(
                        xwc_sb[:m, 0:H], psA[:m, :],
                        mybir.ActivationFunctionType.Copy,
                        scale=dnt[:m, ti : ti + 1],
                    )
                    nc.vector.tensor_scalar(
                        xwc_sb[:m, H : 2 * H], psB[:m, :],
                        dnt[:m, ti : ti + 1], None, mybir.AluOpType.mult,
                    )
                    r0 = ti * _P
                    ri = sum(1 for bb in _BASES[1:] if r0 >= bb)
                    nc.sync.dma_start(
                        t_xwc[ri][r0 - rbounds[ri] : r0 - rbounds[ri] + m, :],
                        xwc_sb[:m, :],
                    )

        for ri in range(_NCLS):
            phase1_range(rbounds[ri] // _P, -(-rbounds[ri + 1] // _P))

        # ---- aggregation: two class rounds, fused z1|z2 ----
        zacc = consts.tile([_P, DT * 2 * H], f32, tag="zacc")
        pos_acc = consts.tile([_P, DT], f32, tag="pos_acc")
        neg_acc = consts.tile([_P, DT], f32, tag="neg_acc")

        cs_ps = sumps.tile([1, H], f32, tag="cs_ps")

        def consume(dti, ps):
            # raw = zacc(lo round) + ps(hi round); z = dinv_d * PReLU(raw)
            #     = (a*dinv_d)*raw + ((1-a)*dinv_d)*max(raw, 0)
            zs = zacc[:, dti * 2 * H : (dti + 1) * 2 * H]
            t0 = misc.tile([_P, 2 * H], f32, tag="t0")
            nc.vector.tensor_tensor(t0[:], zs, ps[:], mybir.AluOpType.add)
            if has_bias:
                nc.vector.tensor_scalar(
                    t0[:], t0[:], ddst[:, dti : dti + 1], None,
                    mybir.AluOpType.mult,
                )
                for h in range(2):
                    nc.vector.tensor_tensor(
                        t0[:, h * H : (h + 1) * H],
                        t0[:, h * H : (h + 1) * H],
                        b_bc[:],
                        mybir.AluOpType.add,
                    )
                t1 = misc.tile([_P, 2 * H], f32, tag="t1")
                nc.vector.tensor_scalar(
                    t1[:], t0[:], 0.0, a_bc[:, 0:1],
                    mybir.AluOpType.min, mybir.AluOpType.mult,
                )
                t2 = misc.tile([_P, 2 * H], f32, tag="t2")
                nc.vector.tensor_scalar(
                    t2[:], t0[:], 0.0, None, mybir.AluOpType.max
                )
                nc.vector.tensor_tensor(zs, t1[:], t2[:], mybir.AluOpType.add)
            else:
                # Relu(c1*t0) = c1*max(t0,0) on the otherwise-idle ACT engine
                t2 = misc.tile([_P, 2 * H], f32, tag="t2")
                nc.scalar.activation(
                    t2[:], t0[:], mybir.ActivationFunctionType.Relu,
                    scale=c1t[:, dti : dti + 1],
                )
                nc.vector.scalar_tensor_tensor(
                    zs, t0[:], adinv[:, dti : dti + 1], t2[:],
                    mybir.AluOpType.mult, mybir.AluOpType.add,
                )
            # accumulate z1 column sums on PE (valid: pad rows are exact 0)
            nc.tensor.matmul(
                cs_ps[:], ones_col[:], zs[:, 0:H],
                start=(dti == 0), stop=(dti == DT - 1),
            )

        # memset gather ring bufs once: slots skipped by trailing -1 indices
        # stay stale; finite data keeps 0*garbage = 0 in PSUM.
        for _ in range(4):
            gz = gpool.tile([_P, maxT, 2 * H], f16, tag="g")
            nc.vector.memset(gz[:, :, :], 0.0)

        for rnd, src_t in enumerate(t_xwc):
            for dti in range(DT):
                m16 = int(Mv[dti, rnd])
                T = -(-m16 // _P)  # tiles actually carrying valid edges
                o = int(Ot[dti, rnd])
                g = gpool.tile([_P, maxT, 2 * H], f16, tag="g")
                nc.gpsimd.dma_gather(
                    g[:, :T, :],
                    src_t[:, :],
                    i_sb[:, 8 * o : 8 * o + m16 // 16],
                    m16,
                    m16,
                    2 * H,
                    single_packet=(m16 <= 1024),
                )
                ps = aggps.tile([_P, 2 * H], f32, tag="aggps")
                for j in range(T):
                    t = o + j
                    eq = stp.tile([_P, _P], f16, tag="eq")
                    nc.vector.tensor_tensor(
                        eq[:],
                        d_sb[:, t : t + 1].to_broadcast([_P, _P]),
                        iota_t[:],
                        mybir.AluOpType.is_equal,
                    )
                    nc.tensor.matmul(
                        ps[:], eq[:], g[:, j, :], start=(j == 0), stop=(j == T - 1)
                    )
                zs = zacc[:, dti * 2 * H : (dti + 1) * 2 * H]
                if rnd == 0:
                    nc.vector.tensor_copy(zs, ps[:])
                elif rnd < _NCLS - 1:
                    nc.vector.tensor_tensor(zs, zs, ps[:], mybir.AluOpType.add)
                else:
                    consume(dti, ps)

        # ---- summary: sigmoid(colsum(z1)/N), AllReduce ----
        cs_sb = misc.tile([1, H], f32, tag="cs_sb")
        nc.vector.tensor_copy(cs_sb[:], cs_ps[:])
        nc.sync.dma_start(t_ar_in[None, :], cs_sb[:])
        nc.gpsimd.collective_compute(
            "AllReduce",
            mybir.AluOpType.add,
            replica_groups=[list(range(C))],
            ins=[t_ar_in[:]],
            outs=[t_ar_out[:]],
        )
        sums_sb = misc.tile([1, H], f32, tag="sums_sb")
        nc.sync.dma_start(sums_sb[:], t_ar_out[None, :])
        summ_sb = misc.tile([1, H], f32, tag="summ_sb")
        nc.scalar.activation(
            summ_sb[:], sums_sb[:], mybir.ActivationFunctionType.Sigmoid,
            scale=1.0 / N,
        )

        # ---- wsum = disc_W @ summary ----
        ident = consts.tile([_P, _P], f32, tag="ident")
        nc.sync.dma_start(ident[:], t_ident[:])
        sT = misc.tile([_P, 2], f32, tag="sT")
        for c_i in range(2):
            tp = miscps.tile([_P, _P], f32, tag="mps")
            nc.tensor.transpose(
                tp[:, 0:1],
                summ_sb[0:1, c_i * _P : (c_i + 1) * _P],
                ident[0:1, 0:1],
            )
            nc.vector.tensor_copy(sT[:, c_i : c_i + 1], tp[:, 0:1])
        ws_ps = miscps.tile([1, H], f32, tag="mps")
        nc.tensor.matmul(ws_ps[:], sT[:, 0:1], dwT0[:], start=True, stop=False)
        nc.tensor.matmul(ws_ps[:], sT[:, 1:2], dwT1[:], start=False, stop=True)
        ws_sb = misc.tile([1, H], f32, tag="ws_sb")
        nc.vector.tensor_copy(ws_sb[:], ws_ps[:])
        wb_ps = miscps.tile([_P, H], f32, tag="mps")
        nc.tensor.matmul(wb_ps[:], ones_row[:], ws_sb[:], start=True, stop=True)
        wsum_bc = consts.tile([_P, H], f32, tag="wsum_bc")
        nc.vector.tensor_copy(wsum_bc[:], wb_ps[:])

        # ---- pos/neg dots (fused multiply + row-sum) ----
        for dti in range(DT):
            for h, acc in ((0, pos_acc), (1, neg_acc)):
                scratch = misc.tile([_P, H], f32, tag="scratch")
                nc.vector.scalar_tensor_tensor(
                    scratch[:],
                    zacc[:, dti * 2 * H + h * H : dti * 2 * H + (h + 1) * H],
                    1.0,
                    wsum_bc[:],
                    mybir.AluOpType.mult,
                    mybir.AluOpType.mult,
                    accum_out=acc[:, dti : dti + 1],
                )

        nc.sync.dma_start(t_pos[:], pos_acc[:])
        nc.sync.dma_start(t_neg[:], neg_acc[:])
        ctx.close()

    nc.compile()

    in_maps = []
    for c in range(C):
        in_maps.append(
            {
                "xcT": xcT,
                "w32": W.astype(np.float32),
                "bvec": b,
                "avec": a,
                "dwT": dwT,
                "iota": iota_np,
                "ident_in": np.eye(_P, dtype=np.float32),
                "dinv_nt": dinv_nt,
                "dinv_dst": dinv_dst[c],
                "adinv_dst": adinv_dst[c],
                "c1_dst": c1_dst[c],
                "idx": i_s[c],
                "dstl": d_s[c],
            }
        )

    if os.environ.get("KERNEL_SIM", "0") == "1":
        from concourse import bass_interp

        sim = bass_interp.MultiCoreSim(nc, C)
        for c in range(C):
            for k, v in in_maps[c].items():
                sim.cores[c].tensor(k)[:] = v
        sim.simulate()
        results = [
            {
                "pos_out": np.array(sim.cores[c].tensor("pos_out")),
                "neg_out": np.array(sim.cores[c].tensor("neg_out")),
            }
            for c in range(C)
        ]
    else:
        trace = os.environ.get("KERNEL_TRACE", "0") == "1"
        kw = {}
        if trace:
            kw["trace"] = True
        res = run_bass_kernel_spmd(nc, in_maps, core_ids=list(range(C)), **kw)
        kernel.last_result = res
        results = res.results

    pos_all = np.stack([results[c]["pos_out"] for c in range(C)])
    neg_all = np.stack([results[c]["neg_out"] for c in range(C)])
    pos = pos_all[coreof, slotof, dtof].astype(np.float32)
    neg = neg_all[coreof, slotof, dtof].astype(np.float32)
    return pos, neg
